# revision 22
# baseline (speedup 1.0000x reference)
"""TRN2 Bass kernel for GPT-2 style causal self-attention (B=4, S=2048, D=1024, H=16).

Sharding: 8 cores = 4 batches x 2 head-groups (8 heads each).
Each core computes qkv projections for its (batch, head-group), runs causal
attention for its 8 heads, computes a partial c_proj, then a pairwise
ReduceScatter (replica groups [[0,1],[2,3],[4,5],[6,7]]) sums the two
head-group partials and splits the token rows between the pair.

All matmuls run in bf16. Softmax needs no max-subtraction (scores bounded
~|2.7| at this scale). Causality is exploited at 128-key-tile granularity:
for diagonal tiles only query columns >= the tile offset are computed and
only the 128-wide partial triangle gets an affine_select.

The AV matmuls for a head pair run concurrently on disjoint PE column
groups (tile_position col tiling, M=64 each), so one PE slot serves both
heads; the softmax denominator is a bf16 running sum of the exp tiles on
the vector engine, partition-reduced by a tiny ones-stationary matmul per
head at the end of each head-pair.

The attention inner loop is one flat software pipeline over (head-pair,
key-tile) with the score+exp stream running SKEW steps ahead of the AV
stream, crossing head-pair boundaries so the ACT engine never drains.
qkv / c_proj work for other chunks is interleaved as filler at a per-chunk
rate; chunk 0's own qkv tail is likewise interleaved into attention(0).
Weights stay resident in SBUF.
"""
import sys
sys.path.insert(0, "/opt/trn_rl_repo")
import numpy as np

B, S, D, H, HD = 4, 2048, 1024, 16, 64
NCORES = 8
HPC = H // 2          # 8 heads per core
ACH = HPC * HD        # 512 local a-channels
P = 128
QCN = 4               # token chunks
QCS = S // QCN        # 512
FKT = D // P          # 8 feature k-tiles
SKEW = 3              # attention pipeline skew (score steps ahead of AV)

_CACHE = {}


def _build():
    from concourse import bacc, tile, mybir
    f32 = mybir.dt.float32
    bf16 = mybir.dt.bfloat16
    Exp = mybir.ActivationFunctionType.Exp

    nc = bacc.Bacc("TRN2", target_bir_lowering=False, debug=False,
                   num_devices=NCORES)
    xt_e = nc.dram_tensor("xt", [D, S], bf16, kind="ExternalInput")
    wq_e = nc.dram_tensor("wq", [D, ACH], bf16, kind="ExternalInput")
    wk_e = nc.dram_tensor("wk", [D, ACH], bf16, kind="ExternalInput")
    wv_e = nc.dram_tensor("wv", [D, ACH], bf16, kind="ExternalInput")
    wp_e = nc.dram_tensor("wp", [ACH, D], bf16, kind="ExternalInput")
    out_e = nc.dram_tensor("outp", [S // 2, D], f32, kind="ExternalOutput")
    rg = [[0, 1], [2, 3], [4, 5], [6, 7]]

    with tile.TileContext(nc) as tc:
        with tc.tile_pool(name="sb", bufs=1) as sb, \
             tc.tile_pool(name="pp", bufs=1, space="PSUM") as pp, \
             tc.tile_pool(name="dr", bufs=1, space="DRAM") as dr:

            kT = [sb.tile([P, S], bf16, name=f"kTr{i}", tag="kT", bufs=4)
                  for i in range(4)]
            VW = HPC * (HD + 1)
            vx = [sb.tile([P, VW], bf16, name=f"vxr{i}", tag="vx", bufs=16)
                  for i in range(16)]
            wv_t = [sb.tile([P, ACH], bf16, name=f"wvr{i}", tag="wv", bufs=8)
                    for i in range(FKT)]
            wp_t = {(a, o): sb.tile([P, 512], bf16, name=f"wpr{a}_{o}",
                                    tag="wp", bufs=8)
                    for a in range(4) for o in range(2)}
            wq_c = [sb.tile([P, FKT, P], bf16, name=f"wqc{ct}", tag="wqc",
                            bufs=4) for ct in range(4)]
            wk_c = [sb.tile([P, FKT, P], bf16, name=f"wkc{ct}", tag="wkc",
                            bufs=4) for ct in range(4)]
            ones_t = sb.tile([P, 1], bf16, name="ones", tag="ones", bufs=1)
            parts = [dr.tile([QCS, D], bf16, name=f"part{q}",
                             tag=f"pq{q}") for q in range(QCN)]
            rsos = [dr.tile([QCS // 2, D], bf16, name=f"rso{q}",
                            tag=f"rq{q}") for q in range(QCN)]

            xc_all = {}    # qc -> xc tiles
            qt_all = {}    # (qc, ct) -> tile
            at_all = {}    # (qc, j) -> tile
            rs_insts = {}

            def qk_units(qc, cts=range(4)):
                """Closures for x-load (cts includes 0) + q/k projections."""
                if 0 in cts:
                    xc = [sb.tile([P, QCS], bf16, name=f"xc{qc}_{k}",
                                  tag="xc", bufs=10) for k in range(FKT)]
                    xc_all[qc] = xc

                    def load_x():
                        for k in range(FKT):
                            nc.sync.dma_start(
                                out=xc[k],
                                in_=xt_e.ap()[k * P:(k + 1) * P,
                                              qc * QCS:(qc + 1) * QCS])
                    yield load_x
                xc = xc_all[qc]
                if qc == 0 and 0 in cts:
                    def load_w0():
                        nc.gpsimd.memset(ones_t, 1.0)
                        for ct in range(4):
                            nc.scalar.dma_start(
                                out=wq_c[ct],
                                in_=wq_e.ap()[:, ct * P:(ct + 1) * P]
                                    .rearrange("(k p) c -> p k c", p=P))
                            nc.scalar.dma_start(
                                out=wk_c[ct],
                                in_=wk_e.ap()[:, ct * P:(ct + 1) * P]
                                    .rearrange("(k p) c -> p k c", p=P))
                        for k in range(FKT):
                            nc.sync.dma_start(
                                out=wv_t[k],
                                in_=wv_e.ap()[k * P:(k + 1) * P, :])
                        for a in range(4):
                            for o in range(2):
                                nc.scalar.dma_start(
                                    out=wp_t[a, o],
                                    in_=wp_e.ap()[a * P:(a + 1) * P,
                                                  o * 512:(o + 1) * 512])
                    yield load_w0
                for ct in cts:
                    for proj, w_c in (("q", wq_c), ("k", wk_c)):
                        mm_ps = pp.tile([P, QCS], f32,
                                        name=f"{proj}ps{qc}_{ct}", tag="mm1",
                                        bufs=2)
                        for k in range(FKT):
                            def mm(k=k, mm_ps=mm_ps, w_ct=w_c[ct], xck=xc[k]):
                                nc.tensor.matmul(mm_ps[:, :], w_ct[:, k, :],
                                                 xck[:, :], start=(k == 0),
                                                 stop=(k == FKT - 1))
                            yield mm
                        if proj == "q":
                            qt = sb.tile([P, QCS], bf16, name=f"qt{qc}_{ct}",
                                         tag="qt", bufs=8)
                            qt_all[qc, ct] = qt

                            def cp(qt=qt, mm_ps=mm_ps):
                                nc.vector.tensor_copy(out=qt, in_=mm_ps)
                            yield cp
                        else:
                            def cp(ct=ct, mm_ps=mm_ps):
                                nc.vector.tensor_copy(
                                    out=kT[ct][:, qc * QCS:(qc + 1) * QCS],
                                    in_=mm_ps)
                            yield cp

            def v_units(qc, vts=range(4)):
                """Closures for the v projection of chunk qc."""
                xc = xc_all[qc]
                for vt in vts:
                    v_ps = pp.tile([P, ACH], f32, name=f"vps{qc}_{vt}",
                                   tag="mm1", bufs=2)
                    for k in range(FKT):
                        def mm(k=k, v_ps=v_ps, xck=xc[k], vt=vt):
                            nc.tensor.matmul(v_ps[:, :],
                                             xck[:, vt * P:(vt + 1) * P],
                                             wv_t[k][:, :], start=(k == 0),
                                             stop=(k == FKT - 1))
                        yield mm

                    def vcp(qc=qc, vt=vt, v_ps=v_ps):
                        vxt = vx[qc * 4 + vt]
                        v3 = vxt.rearrange("p (h w) -> p h w", w=HD + 1)
                        nc.gpsimd.memset(v3[:, :, HD:HD + 1], 1.0)
                        nc.vector.tensor_copy(
                            out=v3[:, :, 0:HD],
                            in_=v_ps.rearrange("p (h d) -> p h d", d=HD))
                    yield vcp

            def cproj_units(qc):
                """Closures for c_proj + RS of qc. at_all is resolved at
                closure-call time (attention(qc) runs before these)."""
                for tt in range(4):
                    for oc in range(2):
                        po = pp.tile([P, 512], f32,
                                     name=f"po{qc}_{tt}_{oc}", tag="mm1",
                                     bufs=2)
                        for a in range(4):
                            def mm(a=a, po=po, tt=tt, oc=oc, qc=qc):
                                nc.tensor.matmul(
                                    po[:, :],
                                    at_all[qc, a][:, tt * P:(tt + 1) * P],
                                    wp_t[a, oc][:, :],
                                    start=(a == 0), stop=(a == 3))
                            yield mm

                        def st_(qc=qc, tt=tt, oc=oc, po=po):
                            pst = sb.tile([P, 512], bf16,
                                          name=f"pst{qc}_{tt}_{oc}",
                                          tag="pst", bufs=4)
                            nc.vector.tensor_copy(out=pst, in_=po)
                            dst = parts[qc][tt * P:(tt + 1) * P,
                                            oc * 512:(oc + 1) * 512]
                            nc.gpsimd.dma_start(out=dst, in_=pst)
                        yield st_

                def rs_(qc=qc):
                    rs_insts[qc] = nc.gpsimd.collective_compute(
                        "ReduceScatter", mybir.AluOpType.add,
                        ins=[parts[qc].opt()],
                        outs=[rsos[qc].opt()],
                        replica_groups=rg)
                yield rs_

            def emit_attention(qc, fillers, rate, skew=SKEW):
                """Flat software pipeline over (head-pair, key-tile); the
                score/exp stream runs `skew` steps ahead of AV. Fillers are
                interleaved between the score and AV halves of each step.
                Emission order is load-bearing: every filler that produces
                an input of a score/AV instruction must be consumed from
                the list before that instruction is emitted, or the PE
                FIFO deadlocks."""
                nkt = 4 * qc + 4
                seq = [(hp, kt) for hp in range(4) for kt in range(nkt)]
                T = len(seq)
                fi = 0
                budget = 0.0
                # bufs=12: at(qc-2) may still be read by a split c_proj
                # while at(qc) is being written
                at_tiles = [sb.tile([P, QCS], bf16, name=f"at{qc}_{j}",
                                    tag="at", bufs=12) for j in range(4)]
                for j in range(4):
                    at_all[qc, j] = at_tiles[j]
                acc = {}
                pts = {}
                for i in range(T + skew):
                    if i < T:
                        hp, kt = seq[i]
                        h_e, h_o = 2 * hp, 2 * hp + 1
                        if kt == 0:
                            for h in (h_e, h_o):
                                acc[h] = pp.tile([65, QCS], f32,
                                                 name=f"acc{qc}_{h}",
                                                 tag="acc", bufs=2)
                        off = max(0, (kt - 4 * qc) * P)
                        # both heads' score tiles share one 2-bank PSUM
                        # tile; a single exp covers the pair
                        st = pp.tile([P, 2 * QCS], f32,
                                     name=f"st{qc}_{hp}_{kt}",
                                     tag="st", bufs=2)
                        for h, half in ((h_e, 0), (h_o, 64)):
                            nc.tensor.matmul(
                                st[:, half * 8 + off:half * 8 + QCS],
                                kT[hp][half:half + 64,
                                       kt * P:(kt + 1) * P],
                                qt_all[qc, hp][half:half + 64, off:],
                                start=True, stop=True,
                                tile_position=(half, 0))
                        pt = sb.tile([P, 2 * QCS], bf16,
                                     name=f"pt{qc}_{hp}_{kt}",
                                     tag="pt", bufs=8)
                        nc.scalar.activation(out=pt, in_=st,
                                             func=Exp, scale=0.125)
                        if kt >= 4 * qc:
                            for half in (0, 64):
                                nc.gpsimd.affine_select(
                                    out=pt[:, half * 8:half * 8 + QCS],
                                    in_=pt[:, half * 8:half * 8 + QCS],
                                    compare_op=mybir.AluOpType.is_ge,
                                    fill=0.0, base=-off,
                                    pattern=[[1, QCS]],
                                    channel_multiplier=-1)
                        pts[hp, kt] = pt
                    budget += rate
                    while fi < len(fillers) and budget >= 1.0:
                        fillers[fi]()
                        fi += 1
                        budget -= 1.0
                    if i >= skew:
                        hp2, kt2 = seq[i - skew]
                        g_e, g_o = 2 * hp2, 2 * hp2 + 1
                        off2 = max(0, (kt2 - 4 * qc) * P)
                        pt2 = pts.pop((hp2, kt2))
                        for h, half in ((g_e, 0), (g_o, 64)):
                            nc.tensor.matmul(
                                acc[h][:, off2:],
                                vx[kt2][:, h * 65:(h + 1) * 65],
                                pt2[:, half * 8 + off2:half * 8 + QCS],
                                start=(kt2 == 0), stop=(kt2 == nkt - 1))
                        if kt2 == nkt - 1:
                            for h, half in ((g_e, 0), (g_o, 64)):
                                # copy the ones-column row to SBUF first:
                                # reciprocal_approx_fast is a custom DVE
                                # bit-op and misreads PSUM on hardware
                                rsum = sb.tile([1, QCS], f32,
                                               name=f"rsum{qc}_{h}",
                                               tag="rs1", bufs=2)
                                nc.vector.tensor_copy(
                                    out=rsum, in_=acc[h][64:65, :])
                                rs_t = sb.tile([1, QCS], f32,
                                               name=f"rst{qc}_{h}",
                                               tag="rs2", bufs=2)
                                nc.vector.reciprocal_approx_fast(
                                    out=rs_t, in_=rsum)
                                rb_t = sb.tile([64, QCS], f32,
                                               name=f"rb{qc}_{h}",
                                               tag="rb", bufs=2)
                                nc.gpsimd.partition_broadcast(
                                    rb_t[:, :], rs_t[:, :])
                                nc.vector.tensor_tensor(
                                    out=at_tiles[hp2][half:half + 64, :],
                                    in0=acc[h][0:64, :], in1=rb_t[:, :],
                                    op=mybir.AluOpType.mult)
                while fi < len(fillers):
                    fillers[fi]()
                    fi += 1

            # PE warmup: dummy matmuls so the HAM clock gate is released
            # before the first real GEMM phase
            wrm = sb.tile([P, QCS], bf16, name="wrm", tag="wrm", bufs=1)
            nc.gpsimd.memset(wrm, 0.0)
            for w in range(16):
                wps = pp.tile([P, QCS], f32, name=f"wps{w}", tag="mm1",
                              bufs=2)
                nc.tensor.matmul(wps[:, :], wrm[:, 0:128], wrm[:, :],
                                 start=True, stop=True)

            # Filler schedule (per-chunk PE slack inside the ACT-bound
            # attention phases):
            #   pre    <- loads, qk(0).ct0
            #   attn0  <- v(0)/qk(0) interleaved JIT, then qk(1)
            #   attn1  <- v(1), qk(2), cproj(0) 1st half
            #   attn2  <- v(2), qk(3), cproj(0) 2nd half, cproj(1)
            #   attn3  <- v(3), cproj(2)
            #   tail   <- cproj(3) + RS(3)
            for u in qk_units(0, cts=[0]):
                u()
            f0 = (list(v_units(0, vts=[0])) + list(qk_units(0, cts=[1]))
                  + list(v_units(0, vts=[1, 2]))
                  + list(qk_units(0, cts=[2]))
                  + list(v_units(0, vts=[3]))
                  + list(qk_units(0, cts=[3])) + list(qk_units(1)))
            cp0 = list(cproj_units(0))
            plans = [
                (f0, 10.0, 6),
                (list(v_units(1)) + list(qk_units(2)) + cp0[:20], 3.7, 3),
                (list(v_units(2)) + list(qk_units(3)) + cp0[20:]
                 + list(cproj_units(1)), 3.4, 3),
                (list(v_units(3)) + list(cproj_units(2)), 3.0, 3),
            ]
            for qc in range(QCN):
                fillers, rate, skew = plans[qc]
                emit_attention(qc, fillers, rate, skew)
            for u in cproj_units(QCN - 1):
                u()

            # final copies of reduced shards (bf16 -> f32 cast DMA) on
            # the gpsimd queue, pinned after the last collective trigger
            # so the scheduler can't hoist their RS-completion waits into
            # the middle of the gpsimd stream
            from concourse.tile import add_dep_helper
            for q in range(QCN):
                di = nc.gpsimd.dma_start(
                    out=out_e.ap()[q * 256:(q + 1) * 256, :],
                    in_=rsos[q][:, :])
                add_dep_helper(di.ins, rs_insts[QCN - 1].ins, sync=False,
                               reason="keep final out DMAs at queue tail")
    nc.compile()
    return nc


def _get_nc():
    if "nc" not in _CACHE:
        _CACHE["nc"] = _build()
    return _CACHE["nc"]


def _in_maps(x, c_attn_w, c_proj_w):
    from ml_dtypes import bfloat16
    maps = []
    for c in range(NCORES):
        b, g = c // 2, c % 2
        h0 = g * HPC
        cols = slice(h0 * HD, h0 * HD + ACH)
        maps.append({
            "xt": np.ascontiguousarray(x[b].T).astype(bfloat16),
            "wq": np.ascontiguousarray(
                c_attn_w[:, :D][:, cols]).astype(bfloat16),
            "wk": np.ascontiguousarray(
                c_attn_w[:, D:2 * D][:, cols]).astype(bfloat16),
            "wv": np.ascontiguousarray(
                c_attn_w[:, 2 * D:][:, cols]).astype(bfloat16),
            "wp": np.ascontiguousarray(
                c_proj_w[h0 * HD:h0 * HD + ACH, :]).astype(bfloat16),
        })
    return maps


def _run(inputs, trace=False):
    from concourse.bass_utils import run_bass_kernel_spmd
    x = np.asarray(inputs["x"], np.float32)
    c_attn_w = np.asarray(inputs["c_attn_w"], np.float32)
    c_attn_b = np.asarray(inputs["c_attn_b"], np.float32)
    c_proj_w = np.asarray(inputs["c_proj_w"], np.float32)
    c_proj_b = np.asarray(inputs["c_proj_b"], np.float32)
    assert not np.any(c_attn_b), "nonzero c_attn_b not supported"

    nc = _get_nc()
    res = run_bass_kernel_spmd(nc, _in_maps(x, c_attn_w, c_proj_w),
                               core_ids=list(range(NCORES)), trace=trace)
    out = np.empty((B, S, D), np.float32)
    for c in range(NCORES):
        b, g = c // 2, c % 2
        o = np.asarray(res.results[c]["outp"], dtype=np.float32)
        for qc in range(QCN):
            tok = qc * QCS + g * 256
            out[b, tok:tok + 256, :] = o[qc * 256:(qc + 1) * 256]
    if np.any(c_proj_b):
        out += c_proj_b
    return out, res


def kernel(**inputs):
    out, _ = _run(inputs, trace=False)
    return out


# revision 24
# speedup vs baseline: 1.1473x; 1.1473x over previous
"""TRN2 Bass kernel for GPT-2 style causal self-attention (B=4, S=2048, D=1024, H=16).

Sharding: 8 cores = 4 batches x 2 head-groups (8 heads each).
Each core computes qkv projections for its (batch, head-group), runs causal
attention for its 8 heads, computes a partial c_proj, then a pairwise
ReduceScatter (replica groups [[0,1],[2,3],[4,5],[6,7]]) sums the two
head-group partials and splits the token rows between the pair.

All matmuls run in bf16. Softmax needs no max-subtraction (scores bounded
~|2.7| at this scale). Causality is exploited at 128-key-tile granularity:
for diagonal tiles only query columns >= the tile offset are computed and
only the 128-wide partial triangle gets an affine_select.

The AV matmuls for a head pair run concurrently on disjoint PE column
groups (tile_position col tiling, M=64 each), so one PE slot serves both
heads; the softmax denominator is a bf16 running sum of the exp tiles on
the vector engine, partition-reduced by a tiny ones-stationary matmul per
head at the end of each head-pair.

The attention inner loop is one flat software pipeline over (head-pair,
key-tile) with the score+exp stream running SKEW steps ahead of the AV
stream, crossing head-pair boundaries so the ACT engine never drains.
qkv / c_proj work for other chunks is interleaved as filler at a per-chunk
rate; chunk 0's own qkv tail is likewise interleaved into attention(0).
Weights stay resident in SBUF.
"""
import sys
sys.path.insert(0, "/opt/trn_rl_repo")
import numpy as np

B, S, D, H, HD = 4, 2048, 1024, 16, 64
NCORES = 8
HPC = H // 2          # 8 heads per core
ACH = HPC * HD        # 512 local a-channels
P = 128
QCN = 4               # token chunks
QCS = S // QCN        # 512
FKT = D // P          # 8 feature k-tiles
SKEW = 3              # attention pipeline skew (score steps ahead of AV)

_CACHE = {}


def _build():
    from concourse import bacc, tile, mybir
    f32 = mybir.dt.float32
    bf16 = mybir.dt.bfloat16
    Exp = mybir.ActivationFunctionType.Exp

    nc = bacc.Bacc("TRN2", target_bir_lowering=False, debug=False,
                   num_devices=NCORES)
    xt_e = nc.dram_tensor("xt", [D, S], bf16, kind="ExternalInput")
    wq_e = nc.dram_tensor("wq", [D, ACH], bf16, kind="ExternalInput")
    wk_e = nc.dram_tensor("wk", [D, ACH], bf16, kind="ExternalInput")
    wv_e = nc.dram_tensor("wv", [D, ACH], bf16, kind="ExternalInput")
    wp_e = nc.dram_tensor("wp", [ACH, D], bf16, kind="ExternalInput")
    out_e = nc.dram_tensor("outp", [S // 2, D], f32, kind="ExternalOutput")
    rg = [[0, 1], [2, 3], [4, 5], [6, 7]]

    with tile.TileContext(nc) as tc:
        with tc.tile_pool(name="sb", bufs=1) as sb, \
             tc.tile_pool(name="pp", bufs=1, space="PSUM") as pp, \
             tc.tile_pool(name="dr", bufs=1, space="DRAM") as dr:

            kT = [sb.tile([P, S], bf16, name=f"kTr{i}", tag="kT", bufs=4)
                  for i in range(4)]
            VW = HPC * (HD + 1)
            vx = [sb.tile([P, VW], bf16, name=f"vxr{i}", tag="vx", bufs=16)
                  for i in range(16)]
            wv_t = [sb.tile([P, ACH], bf16, name=f"wvr{i}", tag="wv", bufs=8)
                    for i in range(FKT)]
            wp_t = {(a, o): sb.tile([P, 512], bf16, name=f"wpr{a}_{o}",
                                    tag="wp", bufs=8)
                    for a in range(4) for o in range(2)}
            wq_c = [sb.tile([P, FKT, P], bf16, name=f"wqc{ct}", tag="wqc",
                            bufs=4) for ct in range(4)]
            wk_c = [sb.tile([P, FKT, P], bf16, name=f"wkc{ct}", tag="wkc",
                            bufs=4) for ct in range(4)]
            ones_t = sb.tile([P, 1], bf16, name="ones", tag="ones", bufs=1)
            parts = [dr.tile([QCS, D], bf16, name=f"part{q}",
                             tag=f"pq{q}") for q in range(QCN)]
            rsos = [dr.tile([QCS // 2, D], bf16, name=f"rso{q}",
                            tag=f"rq{q}") for q in range(QCN)]

            xc_all = {}    # qc -> xc tiles
            qt_all = {}    # (qc, ct) -> tile
            at_all = {}    # (qc, j) -> tile
            rs_insts = {}

            def qk_units(qc, cts=range(4)):
                """Closures for x-load (cts includes 0) + q/k projections."""
                if 0 in cts:
                    xc = [sb.tile([P, QCS], bf16, name=f"xc{qc}_{k}",
                                  tag="xc", bufs=10) for k in range(FKT)]
                    xc_all[qc] = xc

                    def load_x():
                        for k in range(FKT):
                            nc.sync.dma_start(
                                out=xc[k],
                                in_=xt_e.ap()[k * P:(k + 1) * P,
                                              qc * QCS:(qc + 1) * QCS])
                    yield load_x
                xc = xc_all[qc]
                if qc == 0 and 0 in cts:
                    def load_w0():
                        nc.gpsimd.memset(ones_t, 1.0)
                        for ct in range(4):
                            nc.scalar.dma_start(
                                out=wq_c[ct],
                                in_=wq_e.ap()[:, ct * P:(ct + 1) * P]
                                    .rearrange("(k p) c -> p k c", p=P))
                            nc.scalar.dma_start(
                                out=wk_c[ct],
                                in_=wk_e.ap()[:, ct * P:(ct + 1) * P]
                                    .rearrange("(k p) c -> p k c", p=P))
                        for k in range(FKT):
                            nc.sync.dma_start(
                                out=wv_t[k],
                                in_=wv_e.ap()[k * P:(k + 1) * P, :])
                        for a in range(4):
                            for o in range(2):
                                nc.scalar.dma_start(
                                    out=wp_t[a, o],
                                    in_=wp_e.ap()[a * P:(a + 1) * P,
                                                  o * 512:(o + 1) * 512])
                    yield load_w0
                for ct in cts:
                    for proj, w_c in (("q", wq_c), ("k", wk_c)):
                        mm_ps = pp.tile([P, QCS], f32,
                                        name=f"{proj}ps{qc}_{ct}", tag="mm1",
                                        bufs=2)
                        for k in range(FKT):
                            def mm(k=k, mm_ps=mm_ps, w_ct=w_c[ct], xck=xc[k]):
                                nc.tensor.matmul(mm_ps[:, :], w_ct[:, k, :],
                                                 xck[:, :], start=(k == 0),
                                                 stop=(k == FKT - 1))
                            yield mm
                        if proj == "q":
                            qt = sb.tile([P, QCS], bf16, name=f"qt{qc}_{ct}",
                                         tag="qt", bufs=8)
                            qt_all[qc, ct] = qt

                            def cp(qt=qt, mm_ps=mm_ps):
                                nc.vector.tensor_copy(out=qt, in_=mm_ps)
                            yield cp
                        else:
                            def cp(ct=ct, mm_ps=mm_ps):
                                nc.vector.tensor_copy(
                                    out=kT[ct][:, qc * QCS:(qc + 1) * QCS],
                                    in_=mm_ps)
                            yield cp

            def v_units(qc, vts=range(4)):
                """Closures for the v projection of chunk qc."""
                xc = xc_all[qc]
                for vt in vts:
                    v_ps = pp.tile([P, ACH], f32, name=f"vps{qc}_{vt}",
                                   tag="mm1", bufs=2)
                    for k in range(FKT):
                        def mm(k=k, v_ps=v_ps, xck=xc[k], vt=vt):
                            nc.tensor.matmul(v_ps[:, :],
                                             xck[:, vt * P:(vt + 1) * P],
                                             wv_t[k][:, :], start=(k == 0),
                                             stop=(k == FKT - 1))
                        yield mm

                    def vcp(qc=qc, vt=vt, v_ps=v_ps):
                        vxt = vx[qc * 4 + vt]
                        v3 = vxt.rearrange("p (h w) -> p h w", w=HD + 1)
                        nc.gpsimd.memset(v3[:, :, HD:HD + 1], 1.0)
                        nc.vector.tensor_copy(
                            out=v3[:, :, 0:HD],
                            in_=v_ps.rearrange("p (h d) -> p h d", d=HD))
                    yield vcp

            def cproj_units(qc):
                """Closures for c_proj + RS of qc. at_all is resolved at
                closure-call time (attention(qc) runs before these)."""
                for tt in range(4):
                    for oc in range(2):
                        po = pp.tile([P, 512], f32,
                                     name=f"po{qc}_{tt}_{oc}", tag="mm1",
                                     bufs=2)
                        for a in range(4):
                            def mm(a=a, po=po, tt=tt, oc=oc, qc=qc):
                                nc.tensor.matmul(
                                    po[:, :],
                                    at_all[qc, a][:, tt * P:(tt + 1) * P],
                                    wp_t[a, oc][:, :],
                                    start=(a == 0), stop=(a == 3))
                            yield mm

                        def st_(qc=qc, tt=tt, oc=oc, po=po):
                            pst = sb.tile([P, 512], bf16,
                                          name=f"pst{qc}_{tt}_{oc}",
                                          tag="pst", bufs=4)
                            nc.vector.tensor_copy(out=pst, in_=po)
                            dst = parts[qc][tt * P:(tt + 1) * P,
                                            oc * 512:(oc + 1) * 512]
                            nc.gpsimd.dma_start(out=dst, in_=pst)
                        yield st_

                def rs_(qc=qc):
                    rs_insts[qc] = nc.gpsimd.collective_compute(
                        "ReduceScatter", mybir.AluOpType.add,
                        ins=[parts[qc].opt()],
                        outs=[rsos[qc].opt()],
                        replica_groups=rg)
                yield rs_

            def emit_attention(qc, fillers, rate, skew=SKEW):
                """Flat software pipeline over (head-pair, key-tile); the
                score/exp stream runs `skew` steps ahead of AV. Fillers are
                interleaved between the score and AV halves of each step.
                Emission order is load-bearing: every filler that produces
                an input of a score/AV instruction must be consumed from
                the list before that instruction is emitted, or the PE
                FIFO deadlocks."""
                nkt = 4 * qc + 4
                seq = [(hp, kt) for hp in range(4) for kt in range(nkt)]
                T = len(seq)
                fi = 0
                budget = 0.0
                # bufs=12: at(qc-2) may still be read by a split c_proj
                # while at(qc) is being written
                at_tiles = [sb.tile([P, QCS], bf16, name=f"at{qc}_{j}",
                                    tag="at", bufs=12) for j in range(4)]
                for j in range(4):
                    at_all[qc, j] = at_tiles[j]
                acc = {}
                den = {}
                pts = {}
                for i in range(T + skew):
                    if i < T:
                        hp, kt = seq[i]
                        h_e, h_o = 2 * hp, 2 * hp + 1
                        if kt == 0:
                            for h in (h_e, h_o):
                                acc[h] = pp.tile([65, QCS], f32,
                                                 name=f"acc{qc}_{h}",
                                                 tag="acc", bufs=2)
                        off = max(0, (kt - 4 * qc) * P)
                        # both heads' score tiles share one 2-bank PSUM
                        # tile; a single exp covers the pair
                        st = pp.tile([P, 2 * QCS], f32,
                                     name=f"st{qc}_{hp}_{kt}",
                                     tag="st", bufs=2)
                        for h, half in ((h_e, 0), (h_o, 64)):
                            nc.tensor.matmul(
                                st[:, half * 8 + off:half * 8 + QCS],
                                kT[hp][half:half + 64,
                                       kt * P:(kt + 1) * P],
                                qt_all[qc, hp][half:half + 64, off:],
                                start=True, stop=True,
                                tile_position=(half, 0))
                        pt = sb.tile([P, 2 * QCS], bf16,
                                     name=f"pt{qc}_{hp}_{kt}",
                                     tag="pt", bufs=8)
                        nc.scalar.activation(out=pt, in_=st,
                                             func=Exp, scale=0.125)
                        if kt >= 4 * qc:
                            for half in (0, 64):
                                nc.gpsimd.affine_select(
                                    out=pt[:, half * 8:half * 8 + QCS],
                                    in_=pt[:, half * 8:half * 8 + QCS],
                                    compare_op=mybir.AluOpType.is_ge,
                                    fill=0.0, base=-off,
                                    pattern=[[1, QCS]],
                                    channel_multiplier=-1)
                        pts[hp, kt] = pt
                    budget += rate
                    while fi < len(fillers) and budget >= 1.0:
                        fillers[fi]()
                        fi += 1
                        budget -= 1.0
                    if i >= skew:
                        hp2, kt2 = seq[i - skew]
                        g_e, g_o = 2 * hp2, 2 * hp2 + 1
                        off2 = max(0, (kt2 - 4 * qc) * P)
                        pt2 = pts.pop((hp2, kt2))
                        for h, half in ((g_e, 0), (g_o, 64)):
                            nc.tensor.matmul(
                                acc[h][:, off2:],
                                vx[kt2][:, h * 65:(h + 1) * 65],
                                pt2[:, half * 8 + off2:half * 8 + QCS],
                                start=(kt2 == 0), stop=(kt2 == nkt - 1))
                        if kt2 == nkt - 1:
                            for h, half in ((g_e, 0), (g_o, 64)):
                                rsum = sb.tile([1, QCS], f32,
                                               name=f"rsum{qc}_{h}",
                                               tag="rs1", bufs=2)
                                nc.vector.tensor_copy(
                                    out=rsum, in_=acc[h][64:65, :])
                                rs_t = sb.tile([1, QCS], f32,
                                               name=f"rst{qc}_{h}",
                                               tag="rs2", bufs=2)
                                nc.vector.reciprocal_approx_fast(
                                    out=rs_t, in_=rsum)
                                rb_t = sb.tile([64, QCS], f32,
                                               name=f"rb{qc}_{h}",
                                               tag="rb", bufs=2)
                                nc.gpsimd.partition_broadcast(
                                    rb_t[:, :], rs_t[:, :])
                                nc.vector.tensor_tensor(
                                    out=at_tiles[hp2][half:half + 64, :],
                                    in0=acc[h][0:64, :], in1=rb_t[:, :],
                                    op=mybir.AluOpType.mult)
                while fi < len(fillers):
                    fillers[fi]()
                    fi += 1

            # PE warmup: dummy matmuls so the HAM clock gate is released
            # before the first real GEMM phase
            wrm = sb.tile([P, QCS], bf16, name="wrm", tag="wrm", bufs=1)
            nc.gpsimd.memset(wrm, 0.0)
            for w in range(16):
                wps = pp.tile([P, QCS], f32, name=f"wps{w}", tag="mm1",
                              bufs=2)
                nc.tensor.matmul(wps[:, :], wrm[:, 0:128], wrm[:, :],
                                 start=True, stop=True)

            # Filler schedule (per-chunk PE slack inside the ACT-bound
            # attention phases):
            #   pre    <- loads, qk(0).ct0
            #   attn0  <- v(0)/qk(0) interleaved JIT, then qk(1)
            #   attn1  <- v(1), qk(2), cproj(0) 1st half
            #   attn2  <- v(2), qk(3), cproj(0) 2nd half, cproj(1)
            #   attn3  <- v(3), cproj(2)
            #   tail   <- cproj(3) + RS(3)
            for u in qk_units(0, cts=[0]):
                u()
            f0 = (list(v_units(0, vts=[0])) + list(qk_units(0, cts=[1]))
                  + list(v_units(0, vts=[1, 2]))
                  + list(qk_units(0, cts=[2]))
                  + list(v_units(0, vts=[3]))
                  + list(qk_units(0, cts=[3])) + list(qk_units(1)))
            cp0 = list(cproj_units(0))
            plans = [
                (f0, 10.0, 6),
                (list(v_units(1)) + list(qk_units(2)) + cp0[:20], 3.7, 3),
                (list(v_units(2)) + list(qk_units(3)) + cp0[20:]
                 + list(cproj_units(1)), 3.4, 3),
                (list(v_units(3)) + list(cproj_units(2)), 3.0, 3),
            ]
            for qc in range(QCN):
                fillers, rate, skew = plans[qc]
                emit_attention(qc, fillers, rate, skew)
            for u in cproj_units(QCN - 1):
                u()

            # final copies of reduced shards (bf16 -> f32 cast DMA) on
            # the gpsimd queue, pinned after the last collective trigger
            # so the scheduler can't hoist their RS-completion waits into
            # the middle of the gpsimd stream
            from concourse.tile import add_dep_helper
            for q in range(QCN):
                di = nc.gpsimd.dma_start(
                    out=out_e.ap()[q * 256:(q + 1) * 256, :],
                    in_=rsos[q][:, :])
                add_dep_helper(di.ins, rs_insts[QCN - 1].ins, sync=False,
                               reason="keep final out DMAs at queue tail")
    nc.compile()
    return nc


def _get_nc():
    if "nc" not in _CACHE:
        _CACHE["nc"] = _build()
    return _CACHE["nc"]


def _in_maps(x, c_attn_w, c_proj_w):
    from ml_dtypes import bfloat16
    maps = []
    for c in range(NCORES):
        b, g = c // 2, c % 2
        h0 = g * HPC
        cols = slice(h0 * HD, h0 * HD + ACH)
        maps.append({
            "xt": np.ascontiguousarray(x[b].T).astype(bfloat16),
            "wq": np.ascontiguousarray(
                c_attn_w[:, :D][:, cols]).astype(bfloat16),
            "wk": np.ascontiguousarray(
                c_attn_w[:, D:2 * D][:, cols]).astype(bfloat16),
            "wv": np.ascontiguousarray(
                c_attn_w[:, 2 * D:][:, cols]).astype(bfloat16),
            "wp": np.ascontiguousarray(
                c_proj_w[h0 * HD:h0 * HD + ACH, :]).astype(bfloat16),
        })
    return maps


def _run(inputs, trace=False):
    from concourse.bass_utils import run_bass_kernel_spmd
    x = np.asarray(inputs["x"], np.float32)
    c_attn_w = np.asarray(inputs["c_attn_w"], np.float32)
    c_attn_b = np.asarray(inputs["c_attn_b"], np.float32)
    c_proj_w = np.asarray(inputs["c_proj_w"], np.float32)
    c_proj_b = np.asarray(inputs["c_proj_b"], np.float32)
    assert not np.any(c_attn_b), "nonzero c_attn_b not supported"

    nc = _get_nc()
    res = run_bass_kernel_spmd(nc, _in_maps(x, c_attn_w, c_proj_w),
                               core_ids=list(range(NCORES)), trace=trace)
    out = np.empty((B, S, D), np.float32)
    for c in range(NCORES):
        b, g = c // 2, c % 2
        o = np.asarray(res.results[c]["outp"], dtype=np.float32)
        for qc in range(QCN):
            tok = qc * QCS + g * 256
            out[b, tok:tok + 256, :] = o[qc * 256:(qc + 1) * 256]
    if np.any(c_proj_b):
        out += c_proj_b
    return out, res


def kernel(**inputs):
    out, _ = _run(inputs, trace=False)
    return out
